# revision 1
# baseline (speedup 1.0000x reference)
"""DenseGrid trilinear interpolation (embedding_lookup) on 8 trn2 cores.

Strategy: replicate a host-repacked "quad table" of the grid across the
cores and shard the 2M query points. The table row for cell (a, b, c) holds
the 4 corners (a+da, b+db, c) x 12 channels (48 f32 + 16 pad = 256B row);
a dma_gather of elem_size=512B at row (a*H + b)*W + c (elem_step=256B)
fetches rows c and c+1 = all 8 trilinear corners in ONE descriptor/point.

dma_gather indices are int16, so the table is addressed in pages of 32768
rows; the host buckets points by page (and splits each page bucket evenly
across the 8 cores so every core runs the identical NEFF), ships permuted
xyz + prepacked wrapped indices, and un-permutes the result.

Device pipeline per core (~262K + pad points):
  - DVE computes fractional weights from the permuted xyz (bit-matching the
    reference float chain)
  - per page, gpsimd dma_gather pulls 128 f32/point from the table page
  - DVE (+Pool for some corners) does the weighted 8-corner sum, f32
  - result written back in gather order; host scatters to original order
"""

from contextlib import ExitStack

import numpy as np

import concourse.bacc as bacc
import concourse.mybir as mybir
import concourse.tile as tile
from concourse import bass
from concourse.bass_utils import run_bass_kernel_spmd

F32 = mybir.dt.float32
I16 = mybir.dt.int16
I32 = mybir.dt.int32
ALU = mybir.AluOpType

N_CORES = 8
P = 128
ROW = 64          # f32 per table row (48 used + 16 pad) = 256B
ELEM = 2 * ROW    # gathered f32 per point = 512B (rows idx, idx+1)

# corner order matches the reference accumulation order: a=D, b=H, c=W axes,
# c fastest.  Gathered-row offset of corner (da, db, dc):
CORNERS = [(da, db, dc) for da in (0, 1) for db in (0, 1) for dc in (0, 1)]


def corner_off(da, db, dc):
    return dc * ROW + (da * 2 + db) * 12


# ----------------------------------------------------------------- host prep

def build_table(grid, d, h, w, pad_rows=ROW):
    """[1,C,D,H,W] f32 -> [(D*H*W)+pad, ROW] f32 quad table."""
    n_e = d * h * w
    tab = np.zeros((n_e + pad_rows, ROW), np.float32)
    t = tab[:n_e, :48].reshape(d, h, w, 4, 12)
    gt = np.ascontiguousarray(np.transpose(np.asarray(grid[0]), (1, 2, 3, 0)))
    t[:, :, :, 0, :] = gt
    t[:, :-1, :, 1, :] = gt[:, 1:]
    t[:, -1, :, 1, :] = gt[:, -1]
    t[:-1, :, :, 2, :] = gt[1:]
    t[-1, :, :, 2, :] = gt[-1]
    t[:-1, :-1, :, 3, :] = gt[1:, 1:]
    t[:-1, -1, :, 3, :] = gt[1:, -1]
    t[-1, :-1, :, 3, :] = gt[-1, 1:]
    t[-1, -1, :, 3, :] = gt[-1, -1]
    return tab


def prepare(xyz, xyz_min, xyz_max, *, d, h, w, page, n_cores):
    """Normalize, compute cells (bit-matching the device chain), bucket by
    page and split each bucket across cores.

    Returns (per_core_xyzt, per_core_idx, t_list, ids) where ids[c][p] is the
    original point index array for core c / page p (for un-permutation)."""
    n = xyz.shape[0]
    xyzn = ((xyz.astype(np.float32) - xyz_min) /
            (xyz_max - xyz_min)).astype(np.float32)
    # p = v * (dim-1), one f32 rounding — matches device tensor_scalar
    scale = np.array([d - 1, h - 1, w - 1], np.float32)
    pxyz = xyzn * scale
    fl = np.clip(np.floor(pxyz).astype(np.int64), 0,
                 np.array([d - 1, h - 1, w - 1]))
    cell = (fl[:, 0] * h + fl[:, 1]) * w + fl[:, 2]
    pg = (cell // page).astype(np.int64)
    widx = (cell - pg * page).astype(np.int16)
    n_pages = (d * h * w + page - 1) // page

    order = np.argsort(pg, kind="stable")
    counts = np.bincount(pg, minlength=n_pages)
    starts = np.zeros(n_pages + 1, np.int64)
    np.cumsum(counts, out=starts[1:])

    # split page run among cores; shared per-page capacity
    k = np.zeros((n_pages, n_cores), np.int64)
    for c in range(n_cores):
        k[:, c] = counts // n_cores + (c < (counts % n_cores))
    t_list = [int(-(-int(k[p].max()) // P)) for p in range(n_pages)]
    tot_t = sum(t_list)

    per_xyzt, per_idx, ids = [], [], []
    for c in range(n_cores):
        xt = np.zeros((P, tot_t * 3), np.float32)
        ix = np.zeros((P, (tot_t * P) // 16), np.int16)
        idl = []
        toff = 0
        for p in range(n_pages):
            tp = t_list[p]
            if tp == 0:
                idl.append(np.empty(0, np.int64))
                continue
            cp = tp * P
            off = starts[p] + int(k[p, :c].sum())
            my = order[off: off + int(k[p, c])]
            idl.append(my)
            kk = len(my)
            # xyz block [cp, 3] -> [128, tp*3]
            xb = np.zeros((cp, 3), np.float32)
            xb[:kk] = xyzn[my]
            xt[:, toff * 3:(toff + tp) * 3] = (
                xb.reshape(tp, P, 3).transpose(1, 0, 2).reshape(P, tp * 3))
            # idx block wrapped [16, cp/16], replicated to 128 partitions
            wb = np.zeros(cp, np.int16)
            wb[:kk] = widx[my]
            wrap = np.ascontiguousarray(wb.reshape(cp // 16, 16).T)
            ix[:, toff * 8:(toff + tp) * 8] = np.tile(wrap, (8, 1))
            toff += tp
        per_xyzt.append(xt)
        per_idx.append(ix)
        ids.append(idl)
    return xyzn, per_xyzt, per_idx, t_list, ids


def unpermute(results, t_list, ids, n, n_ch=12):
    out = np.empty((n, n_ch), np.float32)
    for c, res in enumerate(results):
        o = res.reshape(P, -1)
        toff = 0
        for p, tp in enumerate(t_list):
            if tp == 0:
                continue
            my = ids[c][p]
            blk = o[:, toff * n_ch:(toff + tp) * n_ch]
            blk = blk.reshape(P, tp, n_ch).transpose(1, 0, 2).reshape(-1, n_ch)
            out[my] = blk[:len(my)]
            toff += tp
    return out


# -------------------------------------------------------------- device kernel

def emit_kernel(tc, out_ap, xyzt_ap, idx_ap, table_ap, *,
                d, h, w, page, t_list, gcols=96, pool_corners=0,
                pool_weights=False, repeat=1, skip_gather=False,
                skip_acc=False, gchunk=0):
    nc = tc.nc
    tot_t = sum(t_list)
    n_pages = len(t_list)
    t_max = max(t_list)

    ctx = ExitStack()
    xt_pool = ctx.enter_context(tc.tile_pool(name="xt", bufs=2))
    fr_pool = ctx.enter_context(tc.tile_pool(name="fr", bufs=1))
    ix_pool = ctx.enter_context(tc.tile_pool(name="ix", bufs=3))
    w_pool = ctx.enter_context(tc.tile_pool(name="w", bufs=2))
    g_pool = ctx.enter_context(tc.tile_pool(name="g", bufs=2))
    acc_pool = ctx.enter_context(tc.tile_pool(name="acc", bufs=2))

    # ---- fractional parts, computed in big batches into resident tiles
    frt = {nm: fr_pool.tile([P, tot_t], F32, tag=f"fr{nm}", name=f"fr{nm}")
           for nm in "abc"}
    n_b = max(1, tot_t // 256)
    bounds = [tot_t * i // n_b for i in range(n_b + 1)]
    for b0, b1 in zip(bounds, bounds[1:]):
        xt = xt_pool.tile([P, (b1 - b0) * 3], F32, tag="xt")
        nc.sync.dma_start(out=xt[:], in_=xyzt_ap[:, b0 * 3:b1 * 3])
        xtv = xt[:].rearrange("p (m c) -> p m c", c=3)
        for ax, (nm, dim) in enumerate((("a", d), ("b", h), ("c", w))):
            src = xtv[:, :, ax]
            px = w_pool.tile([P, b1 - b0], F32, tag="px")
            nc.vector.tensor_scalar(out=px[:], in0=src, scalar1=float(dim - 1),
                                    scalar2=None, op0=ALU.mult)
            ti_ = w_pool.tile([P, b1 - b0], I32, tag="ti")
            nc.vector.tensor_copy(out=ti_[:], in_=px[:])
            if_ = w_pool.tile([P, b1 - b0], F32, tag="tif")
            nc.vector.tensor_copy(out=if_[:], in_=ti_[:])
            fr_ = w_pool.tile([P, b1 - b0], F32, tag="tfr")
            nc.vector.tensor_tensor(out=fr_[:], in0=px[:], in1=if_[:],
                                    op=ALU.subtract)
            m_ = w_pool.tile([P, b1 - b0], F32, tag="tm")
            nc.vector.tensor_scalar(out=m_[:], in0=fr_[:], scalar1=0.0,
                                    scalar2=None, op0=ALU.is_lt)
            nc.vector.tensor_tensor(out=frt[nm][:, b0:b1], in0=fr_[:],
                                    in1=m_[:], op=ALU.add)

    # ---- group pages so accumulation runs on wide tiles
    groups = []
    cur, cw = [], 0
    toff = 0
    soff = 0
    for p in range(n_pages):
        tp = t_list[p]
        if tp and cw + tp > gcols and cur:
            groups.append(cur)
            cur, cw = [], 0
        if tp:
            cur.append((p, tp, toff, soff))
            cw += tp
        toff += tp
        soff += tp * 8
    if cur:
        groups.append(cur)

    out_v = out_ap  # [P, tot_t*12]

    w_eng = nc.gpsimd if pool_weights else nc.vector
    for _ in range(repeat):
        for grp in groups:
            g0 = grp[0][2]
            gc = sum(tp for _, tp, _, _ in grp)

            # weights for this group's columns
            fa, fb, fc = (frt[nm][:, g0:g0 + gc] for nm in "abc")
            w0 = {}
            for nm, f in (("a", fa), ("b", fb), ("c", fc)):
                wt = w_pool.tile([P, gc], F32, tag=f"w0{nm}")
                nc.vector.tensor_scalar(out=wt[:], in0=f, scalar1=-1.0,
                                        scalar2=1.0, op0=ALU.mult, op1=ALU.add)
                w0[nm] = wt[:]
            w1 = {"a": fa, "b": fb, "c": fc}

            def axw(nm, bit):
                return w1[nm] if bit else w0[nm]

            wab = {}
            for da in (0, 1):
                for db in (0, 1):
                    t = w_pool.tile([P, gc], F32, tag=f"wab{da}{db}")
                    w_eng.tensor_tensor(out=t[:], in0=axw("a", da),
                                        in1=axw("b", db), op=ALU.mult)
                    wab[(da, db)] = t
            wk = {}
            for (da, db, dc) in CORNERS:
                t = w_pool.tile([P, gc], F32, tag=f"wk{da}{db}{dc}")
                w_eng.tensor_tensor(out=t[:], in0=wab[(da, db)][:],
                                    in1=axw("c", dc), op=ALU.mult)
                wk[(da, db, dc)] = t

            # gathers for each page in the group
            g = g_pool.tile([P, gc, ELEM], F32, tag="g")
            for (p, tp, ptoff, psoff) in grp:
                cp = tp * P
                if skip_gather:
                    continue
                it = ix_pool.tile([P, cp // 16], I16, tag="ix")
                nc.sync.dma_start(out=it[:],
                                  in_=idx_ap[:, psoff:psoff + tp * 8])
                src = bass.AP(table_ap.tensor, page * ROW * p,
                              [[ROW, page], [1, ELEM]])
                lo = ptoff - g0
                ck = gchunk if gchunk else cp
                for c0 in range(0, cp, ck):
                    c1 = min(c0 + ck, cp)
                    nc.gpsimd.dma_gather(
                        g[:, lo + c0 // P:lo + c1 // P, :], src,
                        it[:, c0 // 16:c1 // 16], c1 - c0, c1 - c0, ELEM,
                        elem_step=ROW, single_packet=(c1 - c0 <= 1024))

            # weighted 8-corner accumulation
            acc = acc_pool.tile([P, gc * 12], F32, tag="acc")
            if skip_acc:
                nc.vector.tensor_copy(out=acc[:, :12], in_=g[:, 0, :12])
                nc.sync.dma_start(out=out_v[:, g0 * 12:(g0 + gc) * 12],
                                  in_=acc[:])
                continue
            accv = acc[:].rearrange("p (m c) -> p m c", c=12)
            tmpv = None
            acc2v = None
            n_dve = 8 - pool_corners
            for ki, (da, db, dc) in enumerate(CORNERS):
                off = corner_off(da, db, dc)
                gk = g[:, :, off:off + 12]
                wb = wk[(da, db, dc)][:].unsqueeze(2).to_broadcast([P, gc, 12])
                if ki < n_dve:
                    if ki == 0:
                        nc.vector.tensor_tensor(out=accv, in0=gk, in1=wb,
                                                op=ALU.mult)
                    else:
                        if tmpv is None:
                            tmp = acc_pool.tile([P, gc * 12], F32, tag="tmp")
                            tmpv = tmp[:].rearrange("p (m c) -> p m c", c=12)
                        nc.vector.tensor_tensor(out=tmpv, in0=gk, in1=wb,
                                                op=ALU.mult)
                        nc.vector.tensor_tensor(out=accv, in0=accv, in1=tmpv,
                                                op=ALU.add)
                else:
                    if acc2v is None:
                        acc2 = acc_pool.tile([P, gc * 12], F32, tag="acc2")
                        acc2v = acc2[:].rearrange("p (m c) -> p m c", c=12)
                        nc.gpsimd.tensor_tensor(out=acc2v, in0=gk, in1=wb,
                                                op=ALU.mult)
                    else:
                        tmp2 = acc_pool.tile([P, gc * 12], F32, tag="tmp2")
                        tmp2v = tmp2[:].rearrange("p (m c) -> p m c", c=12)
                        nc.gpsimd.tensor_tensor(out=tmp2v, in0=gk, in1=wb,
                                                op=ALU.mult)
                        nc.gpsimd.tensor_tensor(out=acc2v, in0=acc2v,
                                                in1=tmp2v, op=ALU.add)
            if acc2v is not None:
                nc.vector.tensor_tensor(out=accv, in0=accv, in1=acc2v,
                                        op=ALU.add)

            nc.sync.dma_start(out=out_v[:, g0 * 12:(g0 + gc) * 12],
                              in_=acc[:])

    ctx.close()


def build_nc(*, d, h, w, page, t_list, n_rows, gcols=96, pool_corners=0,
             pool_weights=False, repeat=1, skip_gather=False, skip_acc=False,
             gchunk=0):
    tot_t = sum(t_list)
    nc = bacc.Bacc("TRN2", target_bir_lowering=False, debug=False)
    xyzt = nc.dram_tensor("xyzt", [P, tot_t * 3], F32,
                          kind="ExternalInput").ap()
    idx = nc.dram_tensor("idx", [P, tot_t * 8], I16,
                         kind="ExternalInput").ap()
    table = nc.dram_tensor("table", [n_rows, ROW], F32,
                           kind="ExternalInput").ap()
    out = nc.dram_tensor("out", [P, tot_t * 12], F32,
                         kind="ExternalOutput").ap()
    with tile.TileContext(nc) as tc:
        emit_kernel(tc, out, xyzt, idx, table, d=d, h=h, w=w, page=page,
                    t_list=t_list, gcols=gcols, pool_corners=pool_corners,
                    pool_weights=pool_weights, repeat=repeat,
                    skip_gather=skip_gather, skip_acc=skip_acc, gchunk=gchunk)
    nc.compile()
    return nc


# ------------------------------------------------------------------- runner

PAGE = 32768


def run(xyz, grid, xyz_min, xyz_max, *, gcols=96, pool_corners=0,
        pool_weights=False, **spmd_kwargs):
    xyz = np.asarray(xyz)
    grid = np.asarray(grid, dtype=np.float32)
    xyz_min = np.asarray(xyz_min, dtype=np.float32)
    xyz_max = np.asarray(xyz_max, dtype=np.float32)

    n = xyz.shape[0]
    _, c, d, h, w = grid.shape
    assert (n, c, d, h, w) == (2097152, 12, 160, 160, 160), (n, c, d, h, w)

    tab = build_table(grid, d, h, w)
    xyzn, per_xyzt, per_idx, t_list, ids = prepare(
        xyz, xyz_min, xyz_max, d=d, h=h, w=w, page=PAGE, n_cores=N_CORES)

    nc = build_nc(d=d, h=h, w=w, page=PAGE, t_list=t_list,
                  n_rows=tab.shape[0], gcols=gcols, pool_corners=pool_corners,
                  pool_weights=pool_weights)
    in_maps = [{"xyzt": per_xyzt[i], "idx": per_idx[i], "table": tab}
               for i in range(N_CORES)]
    res = run_bass_kernel_spmd(nc, in_maps, core_ids=list(range(N_CORES)),
                               **spmd_kwargs)
    out = unpermute([res.results[i]["out"] for i in range(N_CORES)],
                    t_list, ids, n)
    return out, res


def kernel(xyz, grid, xyz_min, xyz_max):
    out, _ = run(xyz, grid, xyz_min, xyz_max)
    return out

